# revision 4
# baseline (speedup 1.0000x reference)
"""ClusterSoftmax (topk_masking) distributed Bass kernel for 8 TRN2 NeuronCores.

Reference semantics (for x >= 0, N = 16777216):
    mask  = x != 0
    e     = where(mask, exp(x), 0)
    denom = sum(e)                # over nonzero entries only
    out   = x * e / denom         # == x * exp(x) / denom  (x==0 rows give 0)

Sharding: x split into 8 contiguous shards of 2M elements, one per core.
Each core computes its local contribution to denom in one streaming pass
(exp with free-axis accumulation on ScalarE + zero-count on VectorE),
all-reduces ONE scalar across the 8 cores, then finishes
out = x * exp(x) * (1/denom) locally. x and exp(x) stay SBUF-resident
between the two phases, so HBM traffic is 8 MiB in + 8 MiB out per core.
"""

import sys

import numpy as np

for _p in ("/root/.axon_site/_ro/trn_rl_repo", "/opt/trn_rl_repo"):
    if _p not in sys.path:
        sys.path.append(_p)

from concourse import bacc, bass, bass_isa, bass_utils, mybir, tile

N = 16777216
NCORES = 8
SHARD = N // NCORES          # 2097152 per core
P = 128                      # SBUF partitions
NT = 8                       # tiles per core
TF = SHARD // (P * NT)       # 2048 free elems per partition per tile
ROWS = NT * P                # 1024 DRAM rows per core

F32 = mybir.dt.float32


def _build():
    nc = bacc.Bacc(
        "TRN2", target_bir_lowering=False, debug=False, num_devices=NCORES
    )
    x_d = nc.dram_tensor("x", [ROWS, TF], F32, kind="ExternalInput")
    o_d = nc.dram_tensor("out", [ROWS, TF], F32, kind="ExternalOutput")

    with tile.TileContext(nc) as tc:
        with (
            tc.tile_pool(name="xp", bufs=NT) as xp,
            tc.tile_pool(name="tp", bufs=NT) as tp,
            tc.tile_pool(name="wp", bufs=3) as wp,
            tc.tile_pool(name="mp", bufs=2) as mp,
            tc.tile_pool(name="sp", bufs=1) as sp,
            tc.tile_pool(name="dp", bufs=1, space="DRAM") as dp,
        ):
            # accumulator columns: [0, NT) = per-partition sums of exp(x)
            # over ALL elements; [NT, 2*NT) = per-partition +count(x == 0).
            # Local denom contribution = sum(cols A) - sum(cols B), since
            # each zero contributes exp(0) = 1 to the exp sum.
            acc = sp.tile([P, 2 * NT], F32, name="acc", tag="acc")

            xs, ts = [], []
            for i in range(NT):
                xt = xp.tile([P, TF], F32, name=f"xt{i}", tag="xt")
                nc.sync.dma_start(out=xt[:], in_=x_d.ap()[i * P:(i + 1) * P, :])
                tt = tp.tile([P, TF], F32, name=f"tt{i}", tag="tt")
                nc.scalar.activation(
                    tt[:], xt[:], mybir.ActivationFunctionType.Exp,
                    accum_out=acc[:, i:i + 1],
                )
                mt = mp.tile([P, TF], F32, name=f"mt{i}", tag="mt")
                # out = (x == 0) as 1.0/0.0; op1 names the accum reduce op
                nc.vector.tensor_scalar(
                    mt[:], xt[:], 0.0, None,
                    mybir.AluOpType.is_equal, mybir.AluOpType.add,
                    accum_out=acc[:, NT + i:NT + i + 1],
                )
                xs.append(xt)
                ts.append(tt)

            # local denom contribution: sum_exp - count_zeros, per partition
            ppa = sp.tile([P, 1], F32, name="ppa", tag="ppa")
            nc.vector.tensor_reduce(
                ppa[:], acc[:, :NT], mybir.AxisListType.X, mybir.AluOpType.add
            )
            ppb = sp.tile([P, 1], F32, name="ppb", tag="ppb")
            nc.vector.tensor_reduce(
                ppb[:], acc[:, NT:], mybir.AxisListType.X, mybir.AluOpType.add
            )
            pp = sp.tile([P, 1], F32, name="pp", tag="pp")
            nc.vector.tensor_tensor(
                pp[:], ppa[:], ppb[:], mybir.AluOpType.subtract
            )
            ppr = sp.tile([P, 1], F32, name="ppr", tag="ppr")
            nc.gpsimd.partition_all_reduce(
                ppr[:], pp[:], P, bass_isa.ReduceOp.add
            )

            # one-scalar AllReduce across the 8 cores (DRAM bounce buffers)
            cin = dp.tile([1, 1], F32, name="cin", tag="cin")
            cout = dp.tile([1, 1], F32, name="cout", tag="cout",
                           addr_space="Shared")
            nc.sync.dma_start(out=cin[:], in_=ppr[0:1, :])
            nc.gpsimd.collective_compute(
                "AllReduce", mybir.AluOpType.add,
                replica_groups=[list(range(NCORES))],
                ins=[cin.opt()], outs=[cout.opt()],
            )
            dsb = sp.tile([1, 1], F32, name="dsb", tag="dsb")
            nc.sync.dma_start(out=dsb[:], in_=cout[:])
            dbc = sp.tile([P, 1], F32, name="dbc", tag="dbc")
            nc.gpsimd.partition_broadcast(dbc[:], dsb[:])
            rsb = sp.tile([P, 1], F32, name="rsb", tag="rsb")
            nc.vector.reciprocal(rsb[:], dbc[:])

            # finish: out = (x * exp(x)) * (1/denom)
            for i in range(NT):
                wt = wp.tile([P, TF], F32, name=f"wt{i}", tag="wt")
                nc.vector.tensor_tensor(
                    wt[:], xs[i][:], ts[i][:], mybir.AluOpType.mult
                )
                yt = xs[i]  # x tile is dead after the multiply; reuse it
                nc.scalar.activation(
                    yt[:], wt[:], mybir.ActivationFunctionType.Copy,
                    0.0, rsb[:],
                )
                nc.sync.dma_start(
                    out=o_d.ap()[i * P:(i + 1) * P, :], in_=yt[:]
                )

    nc.compile()
    return nc


_NC_CACHE = None


def _get_nc():
    global _NC_CACHE
    if _NC_CACHE is None:
        _NC_CACHE = _build()
    return _NC_CACHE


def kernel(x: np.ndarray) -> np.ndarray:
    assert x.shape == (N,) and x.dtype == np.float32
    nc = _get_nc()
    shards = np.ascontiguousarray(x).reshape(NCORES, ROWS, TF)
    in_maps = [{"x": np.ascontiguousarray(shards[i])} for i in range(NCORES)]
    res = bass_utils.run_bass_kernel_spmd(
        nc, in_maps, core_ids=list(range(NCORES))
    )
    out = np.empty((NCORES, ROWS, TF), dtype=np.float32)
    for i in range(NCORES):
        out[i] = res.results[i]["out"]
    return out.reshape(N)


# revision 5
# speedup vs baseline: 1.0498x; 1.0498x over previous
"""ClusterSoftmax (topk_masking) distributed Bass kernel for 8 TRN2 NeuronCores.

Reference semantics (for x >= 0, N = 16777216):
    mask  = x != 0
    e     = where(mask, exp(x), 0)
    denom = sum(e)                # over nonzero entries only
    out   = x * e / denom         # == x * exp(x) / denom  (x==0 rows give 0)

Sharding: x split into 8 contiguous shards of 2M elements, one per core.
Each core computes its local contribution to denom in one streaming pass
(exp with free-axis accumulation on ScalarE + zero-count on VectorE),
all-reduces ONE scalar across the 8 cores, then finishes
out = x * exp(x) * (1/denom) locally. x and exp(x) stay SBUF-resident
between the two phases, so HBM traffic is 8 MiB in + 8 MiB out per core.
"""

import sys

import numpy as np

for _p in ("/root/.axon_site/_ro/trn_rl_repo", "/opt/trn_rl_repo"):
    if _p not in sys.path:
        sys.path.append(_p)

from concourse import bacc, bass, bass_isa, bass_utils, mybir, tile

N = 16777216
NCORES = 8
SHARD = N // NCORES          # 2097152 per core
P = 128                      # SBUF partitions
NT = 8                       # tiles per core
TF = SHARD // (P * NT)       # 2048 free elems per partition per tile
ROWS = NT * P                # 1024 DRAM rows per core

F32 = mybir.dt.float32


def _build():
    nc = bacc.Bacc(
        "TRN2", target_bir_lowering=False, debug=False, num_devices=NCORES
    )
    x_d = nc.dram_tensor("x", [ROWS, TF], F32, kind="ExternalInput")
    o_d = nc.dram_tensor("out", [ROWS, TF], F32, kind="ExternalOutput")

    with tile.TileContext(nc) as tc:
        with (
            tc.tile_pool(name="xp", bufs=NT) as xp,
            tc.tile_pool(name="tp", bufs=NT) as tp,
            tc.tile_pool(name="wp", bufs=3) as wp,
            tc.tile_pool(name="mp", bufs=2) as mp,
            tc.tile_pool(name="sp", bufs=1) as sp,
            tc.tile_pool(name="dp", bufs=1, space="DRAM") as dp,
        ):
            # accumulator columns: [0, NT) = per-partition sums of exp(x)
            # over ALL elements; [NT, 2*NT) = per-partition +count(x == 0).
            # Local denom contribution = sum(cols A) - sum(cols B), since
            # each zero contributes exp(0) = 1 to the exp sum.
            acc = sp.tile([P, 2 * NT], F32, name="acc", tag="acc")

            xs, ts = [], []
            for i in range(NT):
                xt = xp.tile([P, TF], F32, name=f"xt{i}", tag="xt")
                nc.sync.dma_start(out=xt[:], in_=x_d.ap()[i * P:(i + 1) * P, :])
                tt = tp.tile([P, TF], F32, name=f"tt{i}", tag="tt")
                nc.scalar.activation(
                    tt[:], xt[:], mybir.ActivationFunctionType.Exp,
                    accum_out=acc[:, i:i + 1],
                )
                mt = mp.tile([P, TF], F32, name=f"mt{i}", tag="mt")
                # out = (x == 0) as 1.0/0.0; op1 names the accum reduce op
                nc.vector.tensor_scalar(
                    mt[:], xt[:], 0.0, None,
                    mybir.AluOpType.is_equal, mybir.AluOpType.add,
                    accum_out=acc[:, NT + i:NT + i + 1],
                )
                xs.append(xt)
                ts.append(tt)

            # local denom contribution: sum_exp - count_zeros, per partition
            ppa = sp.tile([P, 1], F32, name="ppa", tag="ppa")
            nc.vector.tensor_reduce(
                ppa[:], acc[:, :NT], mybir.AxisListType.X, mybir.AluOpType.add
            )
            ppb = sp.tile([P, 1], F32, name="ppb", tag="ppb")
            nc.vector.tensor_reduce(
                ppb[:], acc[:, NT:], mybir.AxisListType.X, mybir.AluOpType.add
            )
            pp = sp.tile([P, 1], F32, name="pp", tag="pp")
            nc.vector.tensor_tensor(
                pp[:], ppa[:], ppb[:], mybir.AluOpType.subtract
            )
            ppr = sp.tile([P, 1], F32, name="ppr", tag="ppr")
            nc.gpsimd.partition_all_reduce(
                ppr[:], pp[:], P, bass_isa.ReduceOp.add
            )

            # one-scalar AllReduce across the 8 cores (DRAM bounce buffers)
            cin = dp.tile([1, 1], F32, name="cin", tag="cin")
            cout = dp.tile([1, 1], F32, name="cout", tag="cout",
                           addr_space="Shared")
            nc.sync.dma_start(out=cin[:], in_=ppr[0:1, :])
            nc.gpsimd.collective_compute(
                "AllReduce", mybir.AluOpType.add,
                replica_groups=[list(range(NCORES))],
                ins=[cin.opt()], outs=[cout.opt()],
            )
            dsb = sp.tile([1, 1], F32, name="dsb", tag="dsb")
            nc.sync.dma_start(out=dsb[:], in_=cout[:])
            dbc = sp.tile([P, 1], F32, name="dbc", tag="dbc")
            nc.gpsimd.partition_broadcast(dbc[:], dsb[:])
            rsb = sp.tile([P, 1], F32, name="rsb", tag="rsb")
            nc.vector.reciprocal(rsb[:], dbc[:])

            # finish: out = (x * (1/denom)) * exp(x), one fused DVE op/tile
            for i in range(NT):
                yt = wp.tile([P, TF], F32, name=f"yt{i}", tag="yt")
                nc.vector.scalar_tensor_tensor(
                    yt[:], xs[i][:], rsb[:], ts[i][:],
                    mybir.AluOpType.mult, mybir.AluOpType.mult,
                )
                nc.sync.dma_start(
                    out=o_d.ap()[i * P:(i + 1) * P, :], in_=yt[:]
                )

    nc.compile()
    return nc


_NC_CACHE = None


def _get_nc():
    global _NC_CACHE
    if _NC_CACHE is None:
        _NC_CACHE = _build()
    return _NC_CACHE


def kernel(x: np.ndarray) -> np.ndarray:
    assert x.shape == (N,) and x.dtype == np.float32
    nc = _get_nc()
    shards = np.ascontiguousarray(x).reshape(NCORES, ROWS, TF)
    in_maps = [{"x": np.ascontiguousarray(shards[i])} for i in range(NCORES)]
    res = bass_utils.run_bass_kernel_spmd(
        nc, in_maps, core_ids=list(range(NCORES))
    )
    out = np.empty((NCORES, ROWS, TF), dtype=np.float32)
    for i in range(NCORES):
        out[i] = res.results[i]["out"]
    return out.reshape(N)
